# revision 14
# baseline (speedup 1.0000x reference)
"""Single-head causal attention with RoPE on 8 Trainium2 NeuronCores.

Problem: x:(8,2048,1024), Wq/Wk/Wv:(1024,64) -> out:(8,2048,64)
  q = rope(x@Wq); k = rope(x@Wk); v = x@Wv
  out = softmax(causal(q k^T / sqrt(64))) @ v

Sharding: data-parallel over batch B=8, one batch element per core.

v3 restructure (from trace analysis of v2 @ 64us):
  - v2 lost ~13us to HAM half-clock (PE idle gaps during the DMA ramp and
    ACT-paced stretches re-throttled the PE to 1.2GHz) and started phase C
    at 18us because A/V/b for chunks 0+1 were all serialized before it.
  - Per-chunk pipeline: A(0),V(0),b(0) -> C(0) starts ~6us; A/V/b for
    later chunks are REAL fillers drained between C units at ~500ns
    granularity so the PE stays dense (warm) while ACT paces the body.
  - Host-side normalization: the kernel emits [num;den] = [65,512] fp32
    per chunk; the host divides. Kills 16 PE transposes + recip/mul and
    shortens the tail.
  - rot = R2@qk matmul replaced by DVE stream_shuffle (partition swap
    2i<->2i+1) with the sign folded into the sin table.
  - No DMAs issued from the ACT queue (exp is the body pacer).
  - Junk-matmul heartbeat when fillers run dry (late C(3)) to hold HAM
    at 2.4GHz.
"""

import collections
import numpy as np
import ml_dtypes

B, T, C, H = 8, 2048, 1024, 64
NCORES = 8
CHUNK = 512
NCHUNK = T // CHUNK  # 4
NSB = T // 128       # 16 s-blocks
NCB = C // 128       # 8 c-blocks
VHALF = CHUNK // 2   # 256

bf16 = ml_dtypes.bfloat16

import os
JUNK_ON = os.environ.get("K_JUNK", "1") == "1"
SPLIT_COS = os.environ.get("K_SPLIT_COS", "1") == "1"
SYNC_EVICT = os.environ.get("K_SYNC_EVICT", "1") == "1"
USE_SHUFFLE = os.environ.get("K_SHUFFLE", "1") == "1"
MAXPHASE = int(os.environ.get("K_MAXPHASE", "3"))


# ---------------------------------------------------------------- host consts
def _build_consts():
    half = H // 2
    inv_freq = (1.0 / (10000.0 ** (np.arange(half, dtype=np.float32) / half))).astype(
        np.float32
    )
    t = np.arange(T, dtype=np.float32)
    freqs = t[:, None] * inv_freq[None, :]  # (T, half) fp32
    cos = np.repeat(np.cos(freqs), 2, axis=-1)  # (T, H)
    sin = np.repeat(np.sin(freqs), 2, axis=-1)
    cosT = np.ascontiguousarray(cos.T)  # (H, T)
    sinT = np.ascontiguousarray(sin.T)
    # rot via stream_shuffle: rot[2i] = qk[2i+1], rot[2i+1] = qk[2i]; the
    # rope needs rot[2i] = -qk[2i+1], so fold the sign into the sin rows
    sgn = np.where(np.arange(H) % 2 == 0, -1.0, 1.0).astype(np.float32)
    sinTs = sinT * sgn[:, None]

    coscos = np.concatenate([cosT, cosT], axis=0).astype(bf16)  # (128, T)
    sinsin = np.concatenate([sinTs, sinTs], axis=0).astype(bf16)

    sl = np.arange(128)
    trimask = (sl[:, None] <= sl[None, :]).astype(bf16)  # (128, 128)

    identb2 = np.concatenate([np.eye(H), np.eye(H)], axis=0).astype(bf16)  # (128, 64)

    return coscos, sinsin, trimask, identb2


SHUF_MASK = [i ^ 1 for i in range(32)]


# ---------------------------------------------------------------- bass program
def _build_bass():
    import concourse.mybir as mybir
    import concourse.tile as tile
    from concourse import bacc
    from concourse.bass import ts

    BF = mybir.dt.bfloat16
    F32 = mybir.dt.float32
    Exp = mybir.ActivationFunctionType.Exp

    nc = bacc.Bacc(
        "TRN2",
        target_bir_lowering=False,
        debug=False,
        enable_asserts=False,
        num_devices=NCORES,
    )

    # xT prepacked on host to SBUF layout [128(p), chunk, cblk, 512]
    xT_d = nc.dram_tensor("xTp", [128, NCHUNK, NCB, CHUNK], BF, kind="ExternalInput")
    wqk_d = nc.dram_tensor("wqkp", [128, NCB, 128], BF, kind="ExternalInput")
    wv_d = nc.dram_tensor("wvp", [128, NCB, H], BF, kind="ExternalInput")
    coscos_d = nc.dram_tensor("coscos", [128, T], BF, kind="ExternalInput")
    sinsin_d = nc.dram_tensor("sinsin", [128, T], BF, kind="ExternalInput")
    trimask_d = nc.dram_tensor("trimask", [128, 128], BF, kind="ExternalInput")
    identb_d = nc.dram_tensor("identb2", [128, H], BF, kind="ExternalInput")
    # raw [num(64); den(1)] per chunk; host divides
    out_d = nc.dram_tensor("out", [NCHUNK, H + 1, CHUNK], F32, kind="ExternalOutput")

    with tile.TileContext(nc) as tc:
        with (
            tc.tile_pool(name="persist", bufs=1) as persist,
            tc.tile_pool(name="work", bufs=3) as work,
            tc.tile_pool(name="pexpp", bufs=5) as pexpp,
            tc.tile_pool(name="ps_scratch", bufs=2, space="PSUM") as ps_scratch,
            tc.tile_pool(name="ps_sc", bufs=2, space="PSUM") as ps_sc,
            tc.tile_pool(name="ps_out", bufs=2, space="PSUM") as ps_out,
        ):
            # ---- persistent SBUF tensors
            wqk_sb = persist.tile([128, NCB, 128], BF)
            wv_sb = persist.tile([128, NCB, H], BF)
            coscos_sb = persist.tile([128, T], BF)
            sinsin_sb = persist.tile([128, T], BF)
            trimask_sb = persist.tile([128, 128], BF)
            identb_sb = persist.tile([128, H], BF)
            xT_sb = persist.tile([128, NCHUNK, NCB, CHUNK], BF)
            qkrope = persist.tile([128, T], BF)   # q' rows 0:64, k' rows 64:128
            krope0 = persist.tile([H, T], BF)     # k' copy at partitions 0:64
            qrope1 = persist.tile([128, T], BF)   # q' copy at partitions 64:128
            vT_sb = persist.tile([128, NCHUNK, VHALF], BF)  # per-chunk col-halves
            vnat = persist.tile([128, NSB, H + 1], BF)

            # ---- sync HWDGE queue: ordered by need time
            nc.sync.dma_start(out=wqk_sb[:], in_=wqk_d.ap())
            for c2 in range(0, NCB, 2):
                nc.sync.dma_start(
                    out=xT_sb[:, 0, c2 : c2 + 2], in_=xT_d.ap()[:, 0, c2 : c2 + 2]
                )
            if SPLIT_COS:
                nc.sync.dma_start(
                    out=coscos_sb[:, 0:CHUNK], in_=coscos_d.ap()[:, 0:CHUNK]
                )
                nc.sync.dma_start(
                    out=sinsin_sb[:, 0:CHUNK], in_=sinsin_d.ap()[:, 0:CHUNK]
                )
            else:
                nc.sync.dma_start(out=coscos_sb[:], in_=coscos_d.ap())
                nc.sync.dma_start(out=sinsin_sb[:], in_=sinsin_d.ap())
            nc.sync.dma_start(out=wv_sb[:], in_=wv_d.ap())
            for c2 in range(0, NCB, 2):
                nc.sync.dma_start(
                    out=xT_sb[:, 1, c2 : c2 + 2], in_=xT_d.ap()[:, 1, c2 : c2 + 2]
                )
            if SPLIT_COS:
                nc.sync.dma_start(
                    out=coscos_sb[:, CHUNK : 2 * CHUNK],
                    in_=coscos_d.ap()[:, CHUNK : 2 * CHUNK],
                )
                nc.sync.dma_start(
                    out=sinsin_sb[:, CHUNK : 2 * CHUNK],
                    in_=sinsin_d.ap()[:, CHUNK : 2 * CHUNK],
                )
            nc.sync.dma_start(out=xT_sb[:, 2, 0:4], in_=xT_d.ap()[:, 2, 0:4])
            nc.sync.dma_start(out=xT_sb[:, 2, 4:8], in_=xT_d.ap()[:, 2, 4:8])
            if SPLIT_COS:
                nc.sync.dma_start(
                    out=coscos_sb[:, 2 * CHUNK : 3 * CHUNK],
                    in_=coscos_d.ap()[:, 2 * CHUNK : 3 * CHUNK],
                )
                nc.sync.dma_start(
                    out=sinsin_sb[:, 2 * CHUNK : 3 * CHUNK],
                    in_=sinsin_d.ap()[:, 2 * CHUNK : 3 * CHUNK],
                )
            nc.sync.dma_start(out=xT_sb[:, 3], in_=xT_d.ap()[:, 3])
            if SPLIT_COS:
                nc.sync.dma_start(
                    out=coscos_sb[:, 3 * CHUNK :], in_=coscos_d.ap()[:, 3 * CHUNK :]
                )
                nc.sync.dma_start(
                    out=sinsin_sb[:, 3 * CHUNK :], in_=sinsin_d.ap()[:, 3 * CHUNK :]
                )
            # ---- gpsimd (SWDGE): small consts, then per-chunk rope dups
            nc.gpsimd.dma_start(out=identb_sb[:], in_=identb_d.ap())
            nc.gpsimd.dma_start(out=trimask_sb[:], in_=trimask_d.ap())

            # PE warmup: junk matmuls so the HAM clock-gate starts opening
            # while the first DMAs land
            zwarm = persist.tile([128, CHUNK], BF)
            nc.vector.memset(zwarm[:], 0.0)
            nc.vector.memset(vnat[:], 1.0)  # ones col (64); cols 0:64 overwritten
            warm_ps = ps_sc.tile([128, 2, CHUNK], F32, tag="sc", name="warm")
            for w in range(3):
                nc.tensor.matmul(
                    warm_ps[:, 0, :],
                    zwarm[:, 0:128],
                    zwarm[:],
                    start=(w == 0),
                    stop=(w == 2),
                )

            # ---------------- emission units
            qk_tiles = {}

            def emit_qk_alloc(i):
                qk_tiles[i] = ps_scratch.tile(
                    [128, CHUNK], F32, tag="scr", name=f"qk{i}"
                )

            def emit_qk(i, c2):
                # 2 c-blocks of the qk projection accumulation for chunk i
                qk_ps = qk_tiles[i]
                for c in (c2, c2 + 1):
                    nc.tensor.matmul(
                        qk_ps[:],
                        wqk_sb[:, c, :],
                        xT_sb[:, i, c, :],
                        start=(c == 0),
                        stop=(c == NCB - 1),
                        skip_group_check=True,
                    )

            def emit_rotrope(i):
                tsl = ts(i, CHUNK)
                qk_ps = qk_tiles[i]
                qkS = work.tile([128, CHUNK], BF, tag="qkS", name=f"qkS{i}")
                nc.vector.tensor_copy(out=qkS[:], in_=qk_ps[:])
                rotS = work.tile([128, CHUNK], BF, tag="rotS", name=f"rotS{i}")
                if USE_SHUFFLE:
                    nc.vector.stream_shuffle(out=rotS[:], in_=qkS[:], mask=SHUF_MASK)
                else:
                    nc.vector.tensor_copy(out=rotS[:], in_=qkS[:])
                tmp1 = work.tile([128, CHUNK], BF, tag="tmp1", name=f"t1_{i}")
                nc.vector.tensor_mul(tmp1[:], qkS[:], coscos_sb[:, tsl])
                tmp2 = work.tile([128, CHUNK], BF, tag="tmp2", name=f"t2_{i}")
                nc.vector.tensor_mul(tmp2[:], rotS[:], sinsin_sb[:, tsl])
                nc.vector.tensor_add(qkrope[:, tsl], tmp1[:], tmp2[:])
                # partition copies for the two row-tiled score streams
                nc.gpsimd.dma_start(out=krope0[:, tsl], in_=qkrope[H:128, tsl])
                nc.gpsimd.dma_start(out=qrope1[H:128, tsl], in_=qkrope[0:H, tsl])

            v_tiles = {}

            def emit_v_alloc(i):
                v_tiles[i] = ps_scratch.tile(
                    [128, VHALF], F32, tag="scr", name=f"v{i}"
                )

            def emit_v(i, c2):
                # 2 c-blocks of chunk i's v-projection, col-tiled over the
                # chunk's two 256-halves so both PE column groups stream
                # concurrently
                v_ps = v_tiles[i]
                for c in (c2, c2 + 1):
                    nc.tensor.matmul(
                        v_ps[0:H, :],
                        wv_sb[:, c, :],
                        xT_sb[:, i, c, 0:VHALF],
                        start=(c == 0),
                        stop=(c == NCB - 1),
                        skip_group_check=True,
                    )
                    nc.tensor.matmul(
                        v_ps[H:128, :],
                        wv_sb[:, c, :],
                        xT_sb[:, i, c, VHALF:CHUNK],
                        start=(c == 0),
                        stop=(c == NCB - 1),
                        skip_group_check=True,
                    )
                if c2 == NCB - 2:
                    nc.vector.tensor_copy(out=vT_sb[:, i, :], in_=v_ps[:])

            def emit_b(i):
                # transpose chunk i's 4 s-blocks to natural layout; one psum
                # tile per row-group half: the two transpose streams run
                # concurrently and must drain into DIFFERENT banks
                for half_ in range(2):
                    vn_ps = ps_scratch.tile(
                        [128, 2, H], BF, tag="scr", name=f"vn{i}_{half_}"
                    )
                    base = H * half_
                    for j in range(2):
                        nc.tensor.transpose(
                            vn_ps[:, j, :],
                            vT_sb[base : base + H, i, ts(j, 128)],
                            identb_sb[base : base + H, :],
                        )
                    first = 4 * i + 2 * half_
                    nc.vector.tensor_copy(
                        out=vnat[:, first : first + 2, 0:H], in_=vn_ps[:]
                    )

            out_tiles = {}

            def emit_evict(i):
                if os.environ.get("K_NO_EVICT") == "1":
                    return
                out_ps = out_tiles[i]
                outS = work.tile([H + 1, CHUNK], F32, tag="outS", name=f"oS{i}")
                nc.vector.tensor_copy(out=outS[:], in_=out_ps[:])
                if SYNC_EVICT:
                    nc.sync.dma_start(out=out_d.ap()[i], in_=outS[:])
                else:
                    nc.gpsimd.dma_start(out=out_d.ap()[i], in_=outS[:])

            # ---------------- filler queue (cost_ns, key, fn)
            fillers = collections.deque()
            done_keys = set()
            junk_budget = [24 if JUNK_ON else 0]

            def drain_one():
                cost, key, fn = fillers.popleft()
                fn()
                if key is not None:
                    done_keys.add(key)
                return cost

            def drain(budget):
                while fillers and budget > 0:
                    budget -= drain_one()
                # fillers dry: keep the PE dense so HAM holds K=8/8
                while budget > 0 and junk_budget[0] > 0:
                    junk_ps = ps_scratch.tile(
                        [128, CHUNK], F32, tag="scr",
                        name=f"junk{junk_budget[0]}",
                    )
                    nc.tensor.matmul(
                        junk_ps[:], zwarm[:, 0:128], zwarm[:],
                        start=True, stop=True, skip_group_check=True,
                    )
                    junk_budget[0] -= 1
                    budget -= 216

            def drain_until(key):
                if key in done_keys:
                    return
                while fillers:
                    cost, k, fn = fillers[0]
                    drain_one()
                    if k == key:
                        return

            def queue_A(i):
                def qk_unit(c2, first):
                    def fn():
                        if first:
                            emit_qk_alloc(i)
                        emit_qk(i, c2)
                    return fn

                for c2 in range(0, NCB, 2):
                    fillers.append((432, None, qk_unit(c2, c2 == 0)))
                fillers.append((200, ("rotrope", i), lambda: emit_rotrope(i)))

            def queue_V(i):
                def v_unit(c2, first):
                    def fn():
                        if first:
                            emit_v_alloc(i)
                        emit_v(i, c2)
                    return fn

                for c2 in range(0, NCB, 2):
                    fillers.append((110, None, v_unit(c2, c2 == 0)))
                fillers.append((300, ("b", i), lambda: emit_b(i)))

            # ---------------- phase C
            def phase_c(i, diag_pos=None):
                drain_until(("rotrope", i))
                bkey = ("b", i)
                out_ps = ps_out.tile([H + 1, CHUNK], F32, tag="out", name=f"o{i}")
                out_tiles[i] = out_ps
                started = [False]

                punits = [("pair", p) for p in range(2 * i)]
                dunits = [("diag", 0), ("diag", 1)]
                if diag_pos is None:
                    diag_pos = len(punits)
                units = punits[:diag_pos] + dunits + punits[diag_pos:]
                staged = []

                def emit_scores(u):
                    kind, idx = u
                    sc2 = ps_sc.tile(
                        [128, 2, CHUNK], F32, tag="sc", name=f"s{i}_{kind}{idx}"
                    )
                    if kind == "pair":
                        sb = 2 * idx
                        nc.tensor.matmul(
                            sc2[:, 0, :],
                            krope0[:, ts(sb, 128)],
                            qkrope[0:H, ts(i, CHUNK)],
                            start=True,
                            stop=True,
                        )
                        nc.tensor.matmul(
                            sc2[:, 1, :],
                            qkrope[H:128, ts(sb + 1, 128)],
                            qrope1[H:128, ts(i, CHUNK)],
                            start=True,
                            stop=True,
                        )
                    else:
                        j0 = 2 * idx
                        lo0 = 128 * j0
                        nc.tensor.matmul(
                            sc2[:, 0, lo0:CHUNK],
                            krope0[:, ts(4 * i + j0, 128)],
                            qkrope[0:H, i * CHUNK + lo0 : (i + 1) * CHUNK],
                            start=True,
                            stop=True,
                        )
                        # stream B starts at lo0 too: the extra cols ride in
                        # stream A's concurrency shadow and initialize the
                        # region the merged exp reads
                        nc.tensor.matmul(
                            sc2[:, 1, lo0:CHUNK],
                            qkrope[H:128, ts(4 * i + j0 + 1, 128)],
                            qrope1[H:128, i * CHUNK + lo0 : (i + 1) * CHUNK],
                            start=True,
                            stop=True,
                        )
                    staged.append((kind, idx, sc2))

                def emit_exp_num(stage, last_unit):
                    kind, idx, sc2 = stage
                    pexp2 = pexpp.tile(
                        [128, 2, CHUNK], BF, tag="pexp", name=f"p{i}_{kind}{idx}"
                    )
                    if kind == "pair":
                        nc.scalar.activation(
                            out=pexp2[:], in_=sc2[:], func=Exp, scale=0.125
                        )
                        for h_ in range(2):
                            sb = 2 * idx + h_
                            st = not started[0]
                            started[0] = True
                            nc.tensor.matmul(
                                out_ps[:],
                                vnat[:, sb, :],
                                pexp2[:, h_, :],
                                start=st,
                                stop=(last_unit and h_ == 1),
                                skip_group_check=True,
                            )
                    else:
                        j0 = 2 * idx
                        lo0 = 128 * j0
                        nc.scalar.activation(
                            out=pexp2[:, :, lo0:CHUNK],
                            in_=sc2[:, :, lo0:CHUNK],
                            func=Exp,
                            scale=0.125,
                        )
                        # trimask corners on DVE (off PE critical path)
                        for h_ in range(2):
                            lo = 128 * (j0 + h_)
                            nc.vector.tensor_mul(
                                pexp2[:, h_, lo : lo + 128],
                                pexp2[:, h_, lo : lo + 128],
                                trimask_sb[:],
                            )
                        halves = [0, 1]
                        if not started[0]:
                            # a start=True matmul resets the WHOLE psum tile,
                            # so the chunk's first AV must be one full-width
                            # start=True instruction (corner already masked)
                            assert j0 == 0
                            started[0] = True
                            nc.tensor.matmul(
                                out_ps[:],
                                vnat[:, 4 * i, :],
                                pexp2[:, 0, :],
                                start=True,
                                stop=False,
                                skip_group_check=True,
                            )
                            halves = [1]
                        # unmasked AV tails fire straight after exp
                        for h_ in halves:
                            sb = 4 * i + j0 + h_
                            lo = 128 * (j0 + h_)
                            if lo + 128 < CHUNK:
                                nc.tensor.matmul(
                                    out_ps[:, lo + 128 : CHUNK],
                                    vnat[:, sb, :],
                                    pexp2[:, h_, lo + 128 : CHUNK],
                                    start=False,
                                    stop=False,
                                    skip_group_check=True,
                                )
                        # masked corners after the DVE multiplies
                        for h_ in halves:
                            sb = 4 * i + j0 + h_
                            lo = 128 * (j0 + h_)
                            nc.tensor.matmul(
                                out_ps[:, lo : lo + 128],
                                vnat[:, sb, :],
                                pexp2[:, h_, lo : lo + 128],
                                start=False,
                                stop=(last_unit and h_ == 1),
                                skip_group_check=True,
                            )

                first_diag = diag_pos
                for n, u in enumerate(units):
                    if n == first_diag:
                        drain_until(bkey)
                    emit_scores(u)
                    if n > 0:
                        emit_exp_num(staged.pop(0), False)
                    drain(500)
                emit_exp_num(staged.pop(0), True)

            # ---------------- top-level schedule: per-chunk pipeline with
            # A/V/b of later chunks as fillers inside earlier C phases
            if os.environ.get("K_NO_A0") != "1":
                emit_qk_alloc(0)
                for c2 in range(0, NCB, 2):
                    emit_qk(0, c2)
                if os.environ.get("K_NO_ROT0") != "1":
                    emit_rotrope(0)
                done_keys.add(("rotrope", 0))
            if os.environ.get("K_NO_V0") != "1":
                emit_v_alloc(0)
                for c2 in range(0, NCB, 2):
                    emit_v(0, c2)
                if os.environ.get("K_NO_B0") != "1":
                    emit_b(0)
                done_keys.add(("b", 0))

            queue_A(1)
            queue_V(1)
            queue_A(2)
            queue_V(2)
            queue_A(3)
            queue_V(3)

            if os.environ.get("K_NO_C") == "1":
                # emit nothing further: setup + A0/V0/b0 only
                pass
            else:
                phase_c(0)
                emit_evict(0)
            if os.environ.get("K_NO_C") != "1":
                if MAXPHASE >= 1:
                    phase_c(1)
                    emit_evict(1)
                if MAXPHASE >= 2:
                    phase_c(2)
                    emit_evict(2)
                if MAXPHASE >= 3:
                    phase_c(3, diag_pos=2)
                    emit_evict(3)

    nc.compile()
    return nc


_NC_CACHE = None


def _get_nc():
    global _NC_CACHE
    if _NC_CACHE is None:
        _NC_CACHE = _build_bass()
    return _NC_CACHE


def make_in_maps(x, Wq, Wk, Wv):
    """Host-side prep: shard over batch + precompute constants."""
    coscos, sinsin, trimask, identb2 = _build_consts()
    wqk = np.concatenate([Wq, Wk], axis=1).astype(bf16)  # (C, 128)
    wv = Wv.astype(bf16)
    wqkp = np.ascontiguousarray(wqk.reshape(NCB, 128, 128).transpose(1, 0, 2))
    wvp = np.ascontiguousarray(wv.reshape(NCB, 128, H).transpose(1, 0, 2))
    in_maps = []
    for b in range(B):
        xT = x[b].T.astype(bf16)  # (C, T)
        xTp = np.ascontiguousarray(
            xT.reshape(NCB, 128, NCHUNK, CHUNK).transpose(1, 2, 0, 3)
        )
        in_maps.append(
            {
                "xTp": xTp,
                "wqkp": wqkp,
                "wvp": wvp,
                "coscos": coscos,
                "sinsin": sinsin,
                "trimask": trimask,
                "identb2": identb2,
            }
        )
    return in_maps


def postprocess(raw):
    """[NCHUNK, 65, CHUNK] num/den -> (T, H) normalized output."""
    num = raw[:, 0:H, :].astype(np.float32)      # (4, 64, 512)
    den = raw[:, H, :].astype(np.float32)        # (4, 512)
    out = num / den[:, None, :]
    return np.ascontiguousarray(out.transpose(0, 2, 1).reshape(T, H))


def kernel(x, Wq, Wk, Wv):
    from concourse.bass_utils import run_bass_kernel_spmd

    x = np.asarray(x, dtype=np.float32)
    Wq = np.asarray(Wq, dtype=np.float32)
    Wk = np.asarray(Wk, dtype=np.float32)
    Wv = np.asarray(Wv, dtype=np.float32)

    nc = _get_nc()
    in_maps = make_in_maps(x, Wq, Wk, Wv)
    res = run_bass_kernel_spmd(nc, in_maps, core_ids=list(range(NCORES)))
    out = np.stack([postprocess(r["out"]) for r in res.results])  # (B, T, H)
    return np.ascontiguousarray(out.astype(np.float32))
